# revision 12
# baseline (speedup 1.0000x reference)
"""Trainium2 Bass kernel for PhaseCoherenceComputer.

coherence[b,h,q,k] = mean_d cos(phases_q[b,h,q,d] - phases_k[b,h,k,d])
                   = (cos_q @ cos_k^T + sin_q @ sin_k^T) / 64

Shapes: phases_q/k [2, 8, 2048, 64] f32 -> out [2, 8, 2048, 2048] f32.

Strategy (8 NeuronCores, data-parallel over the 16 (b,h) pairs, 2 per core):
- Host ships trig values (rows 0:64 = cos^T, 64:128 = sin^T) as fp8-e4m3
  chunk tensors ordered by need time. fp8 inputs halve the input DMA
  bytes (the kernel is near the aggregate ~240 B/ns DMA ceiling:
  8.4 MB out + 1.05 MB in per core) and add only ~7e-3 normwise error
  (budget: 2e-2; u8 output quantization contributes 6e-3). One K=128
  fp8 matmul per [128 q x 512 k] PSUM slice computes
  cos_q cos_k + sin_q sin_k in a single pass at bf16 PE rate.
- Output is quantized to uint8 during PSUM evacuation (y = x*127 + 128.5)
  and dequantized on host.
- The kernel is paced by the PSUM->SBUF evacuation wall: only DVE and
  ACT can read PSUM, both at 1 elem/cycle/lane for f32 src, so the 8.4M
  output elements per core cost ~37us across both engines (DVE ~1.22us,
  ACT ~1.11us per [128,1024] unit; 30:34 split). Everything else must
  hide under that wall.
- Startup is early-DMA-bandwidth-bound, so the first chunk (q-tile-0
  weights + the first 1024 k columns, one contiguous HBM block) rides
  the fastest-ramping SP ring, with the rest spread over the ACT and
  gpsimd rings; warm-up matmuls on a DVE-memset tile keep the PE busy
  from the first user instruction so the HAM clock gate is released
  (2.4 GHz) early in the real stream. The SP ring carries only that one
  input chunk so the output stream owns it afterwards.
- Output DMAs alternate SP HWDGE / gpsimd SWDGE rings, and the last
  otiles ship per-unit to trim the tail.
"""

import sys

import numpy as np

try:
    import concourse.bacc as bacc
except ImportError:  # fresh interpreter without the axon site path
    for _p in ("/opt/trn_rl_repo", "/root/.axon_site/_ro/trn_rl_repo"):
        if _p not in sys.path:
            sys.path.insert(0, _p)
    import concourse.bacc as bacc

import concourse.mybir as mybir
import concourse.tile as tile
from concourse.bass_utils import run_bass_kernel_spmd

F8 = mybir.dt.float8e4
F32 = mybir.dt.float32
U8 = mybir.dt.uint8

B, H, S, D = 2, 8, 2048, 64
N_CORES = 8
PAIRS_PER_CORE = (B * H) // N_CORES  # 2
Q_TILE = 128
K_TILE = 512
N_QT = S // Q_TILE  # 16
UNIT = 1024  # PSUM unit columns (2 banks)
N_UNITS = S // UNIT  # units per q-tile
N_WARM = 7  # warm-up matmuls (HAM release while inputs stream)
WARM_N = 256  # free dim of warm-up matmuls (short: don't block real MMs)
TAIL_SPLIT = 3  # last otiles shipped per-unit to trim the tail

# Input chunk tensors, ordered by need time. Each is a contiguous
# [128, cols] fp8 block in HBM. SBUF destination column offsets:
#   T[:, 0:128]        u0 q-tile 0       (chunk A head)
#   T[:, 128:2176]     v0 (all 2048 k)   (A tail + B)
#   T[:, 2176:4096]    u0 q-tiles 1..15  (C + D)
#   T[:, 4096:6144]    v1                (G)
#   T[:, 6144:8192]    u1                (H)
CHUNKS = [  # (name, sbuf col offset, cols)
    ("qkA1", 0, 640),
    ("qkA2", 640, 512),
    ("qkB", 1152, 1024),
    ("qkC", 2176, 1024),
    ("qkD", 3200, 896),
    ("qkG", 4096, 2048),
    ("qkH", 6144, 2048),
]
IN_COLS = 8192
V0_OFF = 128
V1_OFF = 4096
U1_OFF = 6144
_NC_CACHE = {}


def _dve_pattern(nd=30, total=64):
    """Evac engine per unit (True=DVE), 64 units per pair-loop cycle.
    30 DVE / 34 ACT: ACT's PSUM reads are ~9% faster, so it takes the
    extra units; the pattern starts A,D so both engines engage on the
    first two units and run gapless to a balanced finish."""
    s, acc = [False, True], 1
    for i in range(2, total):
        nacc = 1 + ((i - 1) * (nd - 1)) // (total - 2)
        s.append(nacc > acc)
        acc = nacc
    return s


def build_kernel():
    """Per-core SPMD program. Inputs: chunk tensors per CHUNKS (fp8 trig
    values packed by need time). Output out [PAIRS, S, S] uint8 with
    x = (u8 - 128) / 127."""
    nc = bacc.Bacc("TRN2", target_bir_lowering=False, debug=False)
    qk = {
        name: nc.dram_tensor(name, [128, cols], F8, kind="ExternalInput")
        for name, _, cols in CHUNKS
    }
    out = nc.dram_tensor("out", [PAIRS_PER_CORE, S, S], U8, kind="ExternalOutput")
    pat = _dve_pattern()

    with tile.TileContext(nc) as tc:
        with (
            tc.tile_pool(name="uv", bufs=1) as uvpool,
            tc.tile_pool(name="wrm", bufs=1) as wpool,
            tc.tile_pool(name="ot", bufs=8) as opool,
            tc.tile_pool(name="psum", bufs=4, space="PSUM") as ppool,
        ):
            T = uvpool.tile([128, IN_COLS], F8, tag="T", name="T")
            warm = wpool.tile([128, WARM_N], F8, tag="w", name="w")

            # Warm-up feed on the (otherwise idle-until-evac) DVE: no DMA
            # dependency, so the PE starts ~0.3us after the preamble.
            nc.vector.memset(warm[:], 0.0)

            # Input DMAs. Ring first-packet ramp: sync ~0.8us,
            # scalar ~1.5-2us, gpsimd ~2.5us. The sync ring carries ONLY
            # the head chunk (the output stream claims it afterwards).
            def chunk_dma(eng, name):
                off, cols = next((o, c) for n, o, c in CHUNKS if n == name)
                eng.dma_start(out=T[:, off : off + cols], in_=qk[name][:, :])

            chunk_dma(nc.sync, "qkA1")  # u0 head + v0[0:512]
            chunk_dma(nc.sync, "qkA2")  # v0[512:1024]
            chunk_dma(nc.scalar, "qkB")  # v0[1024:2048]
            chunk_dma(nc.gpsimd, "qkC")  # u0 q-tiles 1-8
            chunk_dma(nc.scalar, "qkD")  # u0 q-tiles 9-15
            chunk_dma(nc.gpsimd, "qkG")  # v1
            chunk_dma(nc.gpsimd, "qkH")  # u1

            # Warm-up matmuls: release the HAM clock gate while inputs
            # stream. They write a PSUM tile that is recycled by the pool.
            wp = ppool.tile([128, UNIT], F32, tag="ps", name="ps")
            for _ in range(N_WARM):
                nc.tensor.matmul(
                    wp[:, 0:WARM_N],
                    warm[:, 0:128],
                    warm[:, 0:WARM_N],
                    start=True,
                    stop=True,
                )

            def u_slice(p, q):
                if p == 1:
                    return T[:, U1_OFF + q * Q_TILE : U1_OFF + (q + 1) * Q_TILE]
                if q == 0:
                    return T[:, 0:Q_TILE]
                return T[:, 2048 + q * Q_TILE : 2048 + (q + 1) * Q_TILE]

            def v_slice(p, c0, c1):
                off = V1_OFF if p == 1 else V0_OFF
                return T[:, off + c0 : off + c1]

            state = {"u": 0, "ot": 0}
            n_ot = PAIRS_PER_CORE * N_QT

            def q_tile(p, q):
                ot = opool.tile([128, S], U8, tag="ot", name="ot")
                oi = state["ot"]
                state["ot"] += 1
                tail = oi >= n_ot - TAIL_SPLIT
                for un in range(N_UNITS):
                    ps = ppool.tile([128, UNIT], F32, tag="ps", name="ps")
                    for k in range(UNIT // K_TILE):
                        c = un * UNIT + k * K_TILE
                        nc.tensor.matmul(
                            ps[:, k * K_TILE : (k + 1) * K_TILE],
                            u_slice(p, q),
                            v_slice(p, c, c + K_TILE),
                            start=True,
                            stop=True,
                        )
                    i = state["u"]
                    state["u"] += 1
                    osl = ot[:, un * UNIT : (un + 1) * UNIT]
                    if pat[i % len(pat)]:
                        nc.vector.tensor_scalar(
                            osl,
                            ps[:],
                            127.0 / 64.0,
                            128.5,
                            mybir.AluOpType.mult,
                            mybir.AluOpType.add,
                        )
                    else:
                        nc.scalar.activation(
                            osl,
                            ps[:],
                            mybir.ActivationFunctionType.Copy,
                            bias=128.5,
                            scale=127.0 / 64.0,
                        )
                    if tail:
                        # Ship each half as soon as it is evacuated,
                        # alternating rings, to shorten the tail.
                        eng = nc.sync if (2 * oi + un) % 2 == 1 else nc.gpsimd
                        eng.dma_start(
                            out=out[
                                p,
                                q * Q_TILE : (q + 1) * Q_TILE,
                                un * UNIT : (un + 1) * UNIT,
                            ],
                            in_=osl,
                        )
                if not tail:
                    # Alternate output rings to balance NX issue cost.
                    eng = nc.sync if oi % 2 == 0 else nc.gpsimd
                    eng.dma_start(
                        out=out[p, q * Q_TILE : (q + 1) * Q_TILE, :], in_=ot[:]
                    )

            for q in range(N_QT):
                q_tile(0, q)
            for q in range(N_QT):
                q_tile(1, q)
    nc.compile()
    return nc


def _prep_trig(ph):
    """[16, S, D] f32 phases -> [16, 128, S] f64 [cos^T; sin^T]."""
    pht = ph.astype(np.float64).transpose(0, 2, 1)  # [16, D, S]
    return np.concatenate([np.cos(pht), np.sin(pht)], axis=1)


def kernel(phases_q, phases_k, _trace=False):
    import ml_dtypes

    f8 = ml_dtypes.float8_e4m3
    pq = np.asarray(phases_q, dtype=np.float32).reshape(B * H, S, D)
    pk = np.asarray(phases_k, dtype=np.float32).reshape(B * H, S, D)
    qa = _prep_trig(pq)  # [16, 128, S] f64
    ka = _prep_trig(pk)

    in_maps = []
    for c in range(N_CORES):
        p0, p1 = c * PAIRS_PER_CORE, c * PAIRS_PER_CORE + 1
        T = np.empty((128, IN_COLS), dtype=np.float64)
        T[:, 0:128] = qa[p0][:, 0:128]
        T[:, V0_OFF : V0_OFF + S] = ka[p0]
        T[:, 2176:V1_OFF] = qa[p0][:, 128:S]
        T[:, V1_OFF : V1_OFF + S] = ka[p1]
        T[:, U1_OFF : U1_OFF + S] = qa[p1]
        T8 = T.astype(f8)
        in_maps.append(
            {
                name: np.ascontiguousarray(T8[:, off : off + cols])
                for name, off, cols in CHUNKS
            }
        )

    if "nc" not in _NC_CACHE:
        _NC_CACHE["nc"] = build_kernel()
    nc = _NC_CACHE["nc"]

    res = run_bass_kernel_spmd(
        nc, in_maps, core_ids=list(range(N_CORES)), trace=_trace
    )
    full = np.concatenate([r["out"] for r in res.results], axis=0)
    # The f32->u8 cast on device rounds to nearest, so y = x*127 + 128.5
    # lands on round(x*127) + 128.5 +- 0.5; decoding with the same 128.5
    # offset keeps the quantization unbiased.
    out = ((full.astype(np.float32) - 128.5) * (1.0 / 127.0)).reshape(B, H, S, S)
    if _trace:
        return out, res
    return out
